# revision 1
# baseline (speedup 1.0000x reference)
"""2-layer GCN (GCNConv -> BatchNorm(train) -> ReLU -> GCNConv -> ReLU) on 8 TRN2
NeuronCores, SPMD (one NEFF on all cores).

v3 design (from NTFF profiles of v1 fp32 / v2 bf16):
  - bf16 tables / gathers / matmuls (fp32 PSUM + BN stats).
  - Gather calls merged per (super-chunk of 4 dst chunks, src block): 100
    calls/layer instead of 400.  v2 showed the Pool engine's per-call SWDGE
    cost (~1-5us) paced the whole pass while the DMA engines idled at 29%.
    Pads gather row 0 (static counts); at 256B/row the ~20% pad rows cost
    ~3us of engine time - noise.  Scatter keeps 4 open PSUM accumulators
    (one per chunk of the super) since slots are block-major.
  - ONE collective: BN stats ride the AllGather (v1 measured 511us for the
    1KB mesh AllReduce).  The payload is the TRANSPOSED pre-BN agg1 (the
    transposes are needed anyway); after the AllGather each core sums the 8
    stat header rows and builds the full private node-major h2s table with
    just activation+matmul per chunk (no transposes: lhsT IS the shipped
    feature-major layout).
  - dis[src] for layer 2 is folded into the one-hot values; dis[src] for
    layer 1 is folded into the xs table host-side.
  - Edges sorted by (cell, src) so each gather call walks ascending
    addresses (DRAM row-buffer locality).

Sharding: nodes padded 100000 -> 102400 = 8*12800, core i owns rows
[i*12800,(i+1)*12800); edges partitioned by dst owner; weights replicated.
"""
import numpy as np
import ml_dtypes

import concourse.bass as bass
import concourse.mybir as mybir
import concourse.tile as tile
from concourse import bacc
from concourse.bass_utils import run_bass_kernel_spmd
from concourse.masks import make_identity

N = 100000
F = 128
NCORES = 8
NPAD = 102400
OWN = NPAD // NCORES          # 12800
CHUNKS = OWN // 128           # 100
GCHUNKS = NPAD // 128         # 800
NBLK = 4
BLK = NPAD // NBLK            # 25600 (< 32768, int16-addressable)
SEG = OWN + 128               # 12928 AllGather segment rows (stats header)
BN_EPS = 1e-5
SC = 2                        # dst chunks per super-chunk (gather-call merge)
BF16 = ml_dtypes.bfloat16

LAST_EXEC_NS = None
LAST_RESULT = None
_cache = {}


def _prep(x, edge_index):
    src = np.asarray(edge_index[0]).astype(np.int64)
    dst = np.asarray(edge_index[1]).astype(np.int64)

    deg = np.bincount(dst, minlength=N).astype(np.float32) + 1.0
    dis = np.zeros(NPAD, dtype=np.float32)
    dis[:N] = 1.0 / np.sqrt(deg)

    xs = np.zeros((NPAD, F), dtype=np.float32)
    xs[:N] = np.asarray(x, dtype=np.float32) * dis[:N, None]
    xsT = np.ascontiguousarray(xs.T.astype(BF16))          # [F, NPAD] bf16

    owner = dst // OWN
    chunk = (dst % OWN) // 128
    blk = src // BLK
    cell = ((owner * CHUNKS + chunk) * NBLK + blk).astype(np.int64)
    order = np.lexsort((src, cell))      # ascending src within each cell
    src_s = src[order]
    dst_s = dst[order]

    counts = np.zeros((NCORES, CHUNKS, NBLK), np.int64)
    np.add.at(counts, (owner, chunk, blk), 1)
    C = counts.max(axis=0)
    C = ((C + 127) // 128) * 128
    C = np.maximum(C, 128)

    starts = np.zeros(NCORES * CHUNKS * NBLK + 1, dtype=np.int64)
    starts[1:] = np.cumsum(counts.reshape(-1))

    # super-chunk slot layout: for each super s: for each block b: the SC
    # cells (c, b) back to back.  Gather call = one (s, b) segment.
    nsup = CHUNKS // SC
    slot_pos = {}
    sup_meta = []
    off = 0
    for s in range(nsup):
        chs = list(range(s * SC, (s + 1) * SC))
        sup_off = off
        seg_calls = []
        for b in range(NBLK):
            call_off = off
            for c in chs:
                slot_pos[(c, b)] = off
                off += int(C[c, b])
            # split to <=1024 descriptors per call (SWDGE ring limit)
            seg_n = off - call_off
            sub = 0
            while sub < seg_n:
                n = min(1024, seg_n - sub)
                seg_calls.append((b, call_off + sub, n))
                sub += n
        chunk_of = []
        for b in range(NBLK):
            for ci, c in enumerate(chs):
                chunk_of.extend([ci] * (int(C[c, b]) // 128))
        first = {}
        last = {}
        for t, ci in enumerate(chunk_of):
            if ci not in first:
                first[ci] = t
            last[ci] = t
        sup_meta.append({"off": sup_off, "ntiles": len(chunk_of),
                         "chunk_of": chunk_of, "first": first, "last": last,
                         "calls": seg_calls, "chunks": chs})
    tot = off
    ntiles = tot // 128

    per_core = []
    for i in range(NCORES):
        srcidx = np.zeros(tot, dtype=np.int16)                # pads hit row 0
        dstloc = np.full(tot, -1.0, dtype=np.float32)         # pads no column
        dissrc = np.zeros(tot, dtype=np.float32)
        for c in range(CHUNKS):
            for b in range(NBLK):
                k = (i * CHUNKS + c) * NBLK + b
                m = int(counts[i, c, b])
                o = slot_pos[(c, b)]
                if m:
                    sl = slice(starts[k], starts[k] + m)
                    srcidx[o:o + m] = (src_s[sl] - b * BLK).astype(np.int16)
                    dstloc[o:o + m] = (dst_s[sl] % 128).astype(np.float32)
                    dissrc[o:o + m] = dis[src_s[sl]]
        iw = srcidx.reshape(tot // 16, 16).T                  # [16, tot/16]
        srcidx_w = np.ascontiguousarray(np.tile(iw, (8, 1)))  # [128, tot/16]
        dstloc_t = np.ascontiguousarray(
            dstloc.reshape(ntiles, 128).T.astype(BF16))
        dissrc_t = np.ascontiguousarray(
            dissrc.reshape(ntiles, 128).T.astype(BF16))
        disT = np.ascontiguousarray(
            dis[i * OWN:(i + 1) * OWN].reshape(CHUNKS, 128).T)
        xs_ownT = np.ascontiguousarray(xsT[:, i * OWN:(i + 1) * OWN])
        per_core.append({"srcidx": srcidx_w, "dstloc": dstloc_t,
                         "dissrc": dissrc_t, "disT": disT, "xs_ownT": xs_ownT})

    consts = {"tot": tot, "ntiles": ntiles, "sup_meta": sup_meta}
    return consts, xsT, per_core


def _build(consts):
    tot = consts["tot"]
    ntiles = consts["ntiles"]
    sup_meta = consts["sup_meta"]

    f32 = mybir.dt.float32
    bf16 = mybir.dt.bfloat16
    AF = mybir.ActivationFunctionType
    OP = mybir.AluOpType
    nc = bacc.Bacc("TRN2", target_bir_lowering=False, debug=False,
                   num_devices=NCORES, num_swdge_queues=4,
                   dynamic_dma_scratch_size=32768)

    xsT_d = nc.dram_tensor("xsT", [F, NPAD], bf16, kind="ExternalInput").ap()
    xso_d = nc.dram_tensor("xs_ownT", [F, OWN], bf16, kind="ExternalInput").ap()
    W1_d = nc.dram_tensor("W1b", [F, F], bf16, kind="ExternalInput").ap()
    W2_d = nc.dram_tensor("W2b", [F, F], bf16, kind="ExternalInput").ap()
    gamma_d = nc.dram_tensor("gamma_c", [F, 1], f32, kind="ExternalInput").ap()
    beta_d = nc.dram_tensor("beta_c", [F, 1], f32, kind="ExternalInput").ap()
    b2m_d = nc.dram_tensor("b2_mat", [128, F], f32, kind="ExternalInput").ap()
    disT_d = nc.dram_tensor("disT", [128, CHUNKS], f32, kind="ExternalInput").ap()
    srcidx_d = nc.dram_tensor("srcidx", [128, tot // 16], mybir.dt.int16,
                              kind="ExternalInput").ap()
    dstloc_d = nc.dram_tensor("dstloc", [128, ntiles], bf16,
                              kind="ExternalInput").ap()
    dissrc_d = nc.dram_tensor("dissrc", [128, ntiles], bf16,
                              kind="ExternalInput").ap()
    out_d = nc.dram_tensor("out", [OWN, F], f32, kind="ExternalOutput").ap()

    h1s = nc.dram_tensor("h1s_tab", [NPAD, F], bf16)
    h2s = nc.dram_tensor("h2s_tab", [NPAD, F], bf16)
    ag_in = nc.dram_tensor("ag_in", [SEG, F], bf16)
    ag_out = nc.dram_tensor("ag_out", [NCORES * SEG, F], bf16,
                            addr_space="Shared")

    with tile.TileContext(nc) as tc:
        with tc.tile_pool(name="const", bufs=1) as constp, \
             tc.tile_pool(name="big", bufs=1) as bigp, \
             tc.tile_pool(name="xs", bufs=2) as xsp, \
             tc.tile_pool(name="h", bufs=2) as hp, \
             tc.tile_pool(name="psA", bufs=3, space="PSUM") as psA, \
             tc.tile_pool(name="psS", bufs=SC, space="PSUM") as psS, \
             tc.tile_pool(name="pss", bufs=1, space="PSUM") as pss, \
             tc.tile_pool(name="gbuf", bufs=3) as gbufp, \
             tc.tile_pool(name="oh", bufs=6) as ohp, \
             tc.tile_pool(name="wk", bufs=4) as wp, \
             tc.tile_pool(name="st", bufs=1) as stp:

            # ---- constants ----
            W1_t = constp.tile([F, F], bf16)
            W2_t = constp.tile([F, F], bf16)
            ident_f = constp.tile([128, 128], f32)
            iota4 = constp.tile([128, 4, 128], bf16)
            ones_f = constp.tile([128, 1], f32)
            ones_b = constp.tile([128, 1], bf16)
            gamma_t = constp.tile([F, 1], f32)
            beta_t = constp.tile([F, 1], f32)
            b2m_t = constp.tile([128, F], f32)
            disT_t = constp.tile([128, CHUNKS], f32)
            disT2_t = constp.tile([128, CHUNKS], f32)
            nc.sync.dma_start(out=W1_t[:], in_=W1_d[:])
            nc.sync.dma_start(out=W2_t[:], in_=W2_d[:])
            nc.sync.dma_start(out=gamma_t[:], in_=gamma_d[:])
            nc.sync.dma_start(out=beta_t[:], in_=beta_d[:])
            nc.sync.dma_start(out=b2m_t[:], in_=b2m_d[:])
            nc.sync.dma_start(out=disT_t[:], in_=disT_d[:])
            make_identity(nc, ident_f[:])
            iota_i = constp.tile([128, 128], mybir.dt.int32)
            nc.gpsimd.iota(iota_i[:], pattern=[[1, 128]], base=0,
                           channel_multiplier=0)
            for k in range(4):
                nc.vector.tensor_copy(out=iota4[:, k, :], in_=iota_i[:])
            nc.vector.memset(ones_f[:], 1.0)
            nc.vector.memset(ones_b[:], 1.0)
            nc.vector.tensor_tensor(out=disT2_t[:], in0=disT_t[:],
                                    in1=disT_t[:], op=OP.mult)

            srcidx_sb = bigp.tile([128, tot // 16], mybir.dt.int16)
            dstloc_sb = bigp.tile([128, ntiles], bf16)
            dissrc_sb = bigp.tile([128, ntiles], bf16)
            nc.sync.dma_start(out=srcidx_sb[:], in_=srcidx_d[:])
            nc.sync.dma_start(out=dstloc_sb[:], in_=dstloc_d[:])
            nc.sync.dma_start(out=dissrc_sb[:], in_=dissrc_d[:])

            agg = bigp.tile([128, CHUNKS, 128], f32)

            # ---- Phase A: full H1s table (bf16, 16 chunks per DMA group) ----
            BG = 16
            for gg in range(GCHUNKS // BG):
                xs_t = xsp.tile([F, BG * 128], bf16, tag="xs")
                nc.sync.dma_start(
                    out=xs_t[:],
                    in_=xsT_d[:, gg * BG * 128:(gg + 1) * BG * 128])
                hblk = hp.tile([128, BG, F], bf16, tag="h")
                for q in range(BG // 4):
                    ps = psA.tile([128, 4, 128], f32, tag="a")
                    for j in range(4):
                        nc.tensor.matmul(
                            out=ps[:, j, :],
                            lhsT=xs_t[:, (q * 4 + j) * 128:(q * 4 + j + 1) * 128],
                            rhs=W1_t[:], start=True, stop=True)
                    nc.scalar.activation(hblk[:, q * 4:(q + 1) * 4, :], ps[:],
                                         AF.Copy)
                nc.sync.dma_start(
                    out=h1s[gg * BG * 128:(gg + 1) * BG * 128, :]
                        .rearrange("(k p) f -> p k f", p=128),
                    in_=hblk[:])

            # ---- Phase A2: layer-1 self term seeds agg ----
            for q in range(CHUNKS // 4):
                xs_t = xsp.tile([F, 512], bf16, tag="xs")
                nc.sync.dma_start(out=xs_t[:], in_=xso_d[:, q * 512:(q + 1) * 512])
                ps = psA.tile([128, 4, 128], f32, tag="a")
                for j in range(4):
                    nc.tensor.matmul(out=ps[:, j, :],
                                     lhsT=xs_t[:, j * 128:(j + 1) * 128],
                                     rhs=W1_t[:], start=True, stop=True)
                for j in range(4):
                    c = q * 4 + j
                    nc.vector.tensor_scalar_mul(out=agg[:, c, :],
                                                in0=ps[:, j, :],
                                                scalar1=disT_t[:, c:c + 1])

            # BN stat accumulators (separate banks)
            sum_ps = pss.tile([128, 1], f32, name="sum_ps")
            sq_ps = pss.tile([128, 1], f32, name="sq_ps")

            # ---- shared gather/scatter pass (super-chunk granularity) ----
            def layer_pass(table_ap, out_stage, l2):
                qn = 0
                for sm in sup_meta:
                    TS = sm["ntiles"]
                    gb = gbufp.tile([128, TS, 128], bf16, tag="gb")
                    base_t = sm["off"] // 128
                    for (b, coff, n) in sm["calls"]:
                        ol = coff - sm["off"]
                        nc.gpsimd.dma_gather(
                            gb[:, ol // 128:(ol + n) // 128, :],
                            table_ap[b * BLK:(b + 1) * BLK, :],
                            srcidx_sb[:, coff // 16:(coff + n) // 16],
                            n, n, F, queue_num=qn)
                        qn = (qn + 1) % 4
                    accs = [psS.tile([128, F], f32, tag="acc",
                                     name=f"acc{k}")
                            for k in range(SC)]
                    t = 0
                    while t < TS:
                        w = min(4, TS - t)
                        oh = ohp.tile([128, 4, 128], bf16, tag="oh")
                        nc.vector.tensor_tensor(
                            out=oh[:, :w, :],
                            in0=dstloc_sb[:, base_t + t:base_t + t + w]
                                .to_broadcast([128, w, 128]),
                            in1=iota4[:, :w, :], op=OP.is_equal)
                        if l2:
                            nc.vector.tensor_tensor(
                                out=oh[:, :w, :], in0=oh[:, :w, :],
                                in1=dissrc_sb[:, base_t + t:base_t + t + w]
                                    .to_broadcast([128, w, 128]),
                                op=OP.mult)
                        for j in range(w):
                            ci = sm["chunk_of"][t + j]
                            nc.tensor.matmul(out=accs[ci][:],
                                             lhsT=oh[:, j, :],
                                             rhs=gb[:, t + j, :],
                                             start=(sm["first"][ci] == t + j),
                                             stop=(sm["last"][ci] == t + j))
                        t += w
                    for ci, c in enumerate(sm["chunks"]):
                        out_stage(c, accs[ci])

            # ---- L1 scatter: agg += dis_dst * ps; BN stats ride along
            #      (hidden under the DMA-bound gather pass) ----
            def b_stage(c, ps):
                tt = wp.tile([128, 128], f32, tag="bs")
                nc.vector.tensor_scalar_mul(out=tt[:], in0=ps[:],
                                            scalar1=disT_t[:, c:c + 1])
                nc.vector.tensor_tensor(out=agg[:, c, :], in0=tt[:],
                                        in1=agg[:, c, :], op=OP.add)
                nc.tensor.matmul(out=sum_ps[:], lhsT=agg[:, c, :],
                                 rhs=ones_f[:],
                                 start=(c == 0), stop=(c == CHUNKS - 1))
                sq = wp.tile([128, 128], bf16, tag="sq")
                nc.scalar.square(sq[:], agg[:, c, :])
                nc.tensor.matmul(out=sq_ps[:], lhsT=sq[:], rhs=ones_b[:],
                                 start=(c == 0), stop=(c == CHUNKS - 1))
            layer_pass(h1s.ap(), b_stage, l2=False)

            # ---- stats header -> ag_in rows [0,128) (rows 0,1 used) ----
            stats2 = stp.tile([128, 2], f32)
            nc.vector.tensor_copy(out=stats2[:, 0:1], in_=sum_ps[:])
            nc.vector.tensor_copy(out=stats2[:, 1:2], in_=sq_ps[:])
            stpad = stp.tile([128, 128], f32)
            nc.vector.memset(stpad[:], 0.0)
            nc.vector.tensor_copy(out=stpad[:, 0:2], in_=stats2[:])
            trs = psA.tile([128, 4, 128], f32, tag="a")
            nc.tensor.transpose(out=trs[:, 0, :], in_=stpad[:],
                                identity=ident_f[:])
            stag = stp.tile([128, 128], bf16)
            nc.scalar.activation(stag[:], trs[:, 0, :], AF.Copy)
            nc.sync.dma_start(out=ag_in[0:128, :], in_=stag[:])

            # ---- transpose agg1 (needed for layer 2 anyway), ship agg1^T ----
            for q in range(CHUNKS // 4):
                trp = psA.tile([128, 4, 128], f32, tag="a")
                for j in range(4):
                    nc.tensor.transpose(out=trp[:, j, :],
                                        in_=agg[:, q * 4 + j, :],
                                        identity=ident_f[:])
                tst = wp.tile([128, 4, 128], bf16, tag="tT")
                nc.scalar.activation(tst[:], trp[:], AF.Copy)
                nc.sync.dma_start(
                    out=ag_in[128 + q * 512:128 + (q + 1) * 512, :]
                        .rearrange("(k p) f -> p k f", p=128),
                    in_=tst[:])

            nc.gpsimd.collective_compute(
                "AllGather", OP.bypass, ins=[ag_in.ap()], outs=[ag_out.ap()],
                replica_groups=[list(range(NCORES))])

            # ---- global BN stats from the 8 headers ----
            # (two plain DMAs: a partition-dim rearrange on the SBUF side of
            # a DMA silently misplaces rows - learned the hard way)
            gst = stp.tile([16, 128], bf16)
            agv = ag_out.ap().rearrange("(i s) f -> i s f", i=NCORES)
            nc.sync.dma_start(out=gst[0:8, :], in_=agv[:, 0, :])
            nc.sync.dma_start(out=gst[8:16, :], in_=agv[:, 1, :])
            gpad = stp.tile([128, 128], f32)
            nc.vector.memset(gpad[:], 0.0)
            nc.vector.tensor_copy(out=gpad[0:16, :], in_=gst[:])
            gtr = psA.tile([128, 4, 128], f32, tag="a")
            nc.tensor.transpose(out=gtr[:, 0, :], in_=gpad[:],
                                identity=ident_f[:])
            # cols 0..7 = per-core sums, 8..15 = per-core sumsqs
            gred = stp.tile([128, 16], f32)
            nc.vector.tensor_copy(out=gred[:], in_=gtr[:, 0, 0:16])
            nc.vector.tensor_tensor(out=gred[:, 0:4], in0=gred[:, 0:4],
                                    in1=gred[:, 4:8], op=OP.add)
            nc.vector.tensor_tensor(out=gred[:, 8:12], in0=gred[:, 8:12],
                                    in1=gred[:, 12:16], op=OP.add)
            nc.vector.tensor_tensor(out=gred[:, 0:2], in0=gred[:, 0:2],
                                    in1=gred[:, 2:4], op=OP.add)
            nc.vector.tensor_tensor(out=gred[:, 8:10], in0=gred[:, 8:10],
                                    in1=gred[:, 10:12], op=OP.add)
            nc.vector.tensor_tensor(out=gred[:, 0:1], in0=gred[:, 0:1],
                                    in1=gred[:, 1:2], op=OP.add)
            nc.vector.tensor_tensor(out=gred[:, 8:9], in0=gred[:, 8:9],
                                    in1=gred[:, 9:10], op=OP.add)

            mean_t = stp.tile([128, 1], f32)
            ex2_t = stp.tile([128, 1], f32)
            var_t = stp.tile([128, 1], f32)
            sd_t = stp.tile([128, 1], f32)
            rstd_t = stp.tile([128, 1], f32)
            scale_c = stp.tile([128, 1], f32)
            shift_c = stp.tile([128, 1], f32)
            eps_t = stp.tile([128, 1], f32)
            nc.vector.tensor_scalar_mul(out=mean_t[:], in0=gred[:, 0:1],
                                        scalar1=1.0 / N)
            nc.vector.tensor_scalar_mul(out=ex2_t[:], in0=gred[:, 8:9],
                                        scalar1=1.0 / N)
            nc.vector.tensor_tensor(out=var_t[:], in0=mean_t[:], in1=mean_t[:],
                                    op=OP.mult)
            nc.vector.tensor_tensor(out=var_t[:], in0=ex2_t[:], in1=var_t[:],
                                    op=OP.subtract)
            nc.vector.tensor_scalar_max(out=var_t[:], in0=var_t[:],
                                        scalar1=0.0)
            nc.vector.memset(eps_t[:], BN_EPS)
            nc.scalar.activation(sd_t[:], var_t[:], AF.Sqrt, bias=eps_t[:])
            nc.vector.reciprocal(out=rstd_t[:], in_=sd_t[:])
            nc.vector.tensor_tensor(out=scale_c[:], in0=rstd_t[:],
                                    in1=gamma_t[:], op=OP.mult)
            nc.vector.tensor_tensor(out=shift_c[:], in0=mean_t[:],
                                    in1=scale_c[:], op=OP.mult)
            nc.vector.tensor_tensor(out=shift_c[:], in0=beta_t[:],
                                    in1=shift_c[:], op=OP.subtract)

            # ---- layer-2 self-term seeds (own agg1^T read back from the
            #      private ag_in copy; per-core static address) ----
            for q in range(CHUNKS // 4):
                stb = xsp.tile([128, 4, 128], bf16, tag="cb")
                nc.sync.dma_start(
                    out=stb[:],
                    in_=ag_in.ap()[128 + q * 512:128 + (q + 1) * 512, :]
                        .rearrange("(k p) f -> p k f", p=128))
                h2in = wp.tile([128, 4, 128], bf16, tag="h2")
                nc.scalar.activation(h2in[:], stb[:],
                                     AF.Relu, bias=shift_c[:], scale=scale_c[:])
                ps2 = psA.tile([128, 4, 128], f32, tag="a")
                for j in range(4):
                    nc.tensor.matmul(out=ps2[:, j, :], lhsT=h2in[:, j, :],
                                     rhs=W2_t[:], start=True, stop=True)
                for j in range(4):
                    c = q * 4 + j
                    nc.vector.tensor_scalar_mul(out=agg[:, c, :],
                                                in0=ps2[:, j, :],
                                                scalar1=disT2_t[:, c:c + 1])
                    nc.vector.tensor_tensor(out=agg[:, c, :], in0=agg[:, c, :],
                                            in1=b2m_t[:], op=OP.add)

            # ---- build full private h2s table from ag_out (feature-major
            #      payload: no transposes needed) ----
            for i in range(NCORES):
                for g0 in range(0, CHUNKS, 4):
                    ctb = xsp.tile([128, 4, 128], bf16, tag="cb")
                    rows0 = i * SEG + 128 + g0 * 128
                    nc.sync.dma_start(
                        out=ctb[:],
                        in_=ag_out.ap()[rows0:rows0 + 512, :]
                            .rearrange("(k p) f -> p k f", p=128))
                    h2in = wp.tile([128, 4, 128], bf16, tag="h2")
                    nc.scalar.activation(h2in[:], ctb[:], AF.Relu,
                                         bias=shift_c[:], scale=scale_c[:])
                    ps2 = psA.tile([128, 4, 128], f32, tag="a")
                    for j in range(4):
                        nc.tensor.matmul(out=ps2[:, j, :], lhsT=h2in[:, j, :],
                                         rhs=W2_t[:], start=True, stop=True)
                    hb2 = hp.tile([128, 4, F], bf16, tag="h")
                    nc.vector.tensor_copy(out=hb2[:], in_=ps2[:])
                    orow = i * OWN + g0 * 128
                    nc.sync.dma_start(
                        out=h2s[orow:orow + 512, :]
                            .rearrange("(k p) f -> p k f", p=128),
                        in_=hb2[:])

            # ---- layer-2 scatter + relu + output ----
            def e_stage(c, ps):
                tt = wp.tile([128, 128], f32, tag="eo")
                nc.vector.tensor_scalar_mul(out=tt[:], in0=ps[:],
                                            scalar1=disT_t[:, c:c + 1])
                nc.vector.tensor_tensor(out=tt[:], in0=tt[:], in1=agg[:, c, :],
                                        op=OP.add)
                ot = wp.tile([128, 128], f32, tag="ot")
                nc.scalar.activation(ot[:], tt[:], AF.Relu)
                nc.sync.dma_start(out=out_d[c * 128:(c + 1) * 128, :], in_=ot[:])
            layer_pass(h2s.ap(), e_stage, l2=True)

    nc.compile()
    return nc


def kernel(**inputs):
    global LAST_EXEC_NS, LAST_RESULT
    import os
    x = inputs["x"]
    W1 = np.asarray(inputs["W1"], dtype=np.float32)
    W2 = np.asarray(inputs["W2"], dtype=np.float32)
    gamma = np.asarray(inputs["gamma"], dtype=np.float32)
    beta = np.asarray(inputs["beta"], dtype=np.float32)
    b2 = np.asarray(inputs["b2"], dtype=np.float32)
    edge_index = inputs["edge_index"]

    key = (hash(np.asarray(edge_index)[:, ::997].tobytes()),)
    if key not in _cache:
        consts, xsT, per_core = _prep(x, edge_index)
        nc = _build(consts)
        _cache[key] = (consts, nc)
    else:
        consts, nc = _cache[key]
        _, xsT, per_core = _prep(x, edge_index)

    shared = {
        "xsT": xsT,
        "W1b": W1.astype(BF16), "W2b": W2.astype(BF16),
        "gamma_c": gamma.reshape(F, 1).copy(),
        "beta_c": beta.reshape(F, 1).copy(),
        "b2_mat": np.ascontiguousarray(np.broadcast_to(b2.reshape(1, F),
                                                       (128, F))),
    }
    in_maps = []
    for i in range(NCORES):
        m = dict(shared)
        m.update(per_core[i])
        in_maps.append(m)

    trace = bool(os.environ.get("BASS_GCN_TRACE"))
    res = run_bass_kernel_spmd(nc, in_maps, list(range(NCORES)), trace=trace)
    LAST_EXEC_NS = res.exec_time_ns
    LAST_RESULT = res

    out = np.concatenate([res.results[i]["out"] for i in range(NCORES)], axis=0)
    return np.ascontiguousarray(out[:N]).astype(np.float32)



# revision 10
# speedup vs baseline: 1.2765x; 1.2765x over previous
"""2-layer GCN (GCNConv -> BatchNorm(train) -> ReLU -> GCNConv -> ReLU) on 8 TRN2
NeuronCores, SPMD (one NEFF on all cores).

v4 design (from the v3 NTFF profile: GpSimd 65% busy all in DMAGatherAnt,
DMA engines ~45%, phase A + h2s-build windows serial):
  - W applied AFTER aggregation (matmul commutes with the scatter-sum):
    L1 gathers raw xs = x*dis rows from a host-shipped node-major table, so
    the per-core "h1s" table build (52MB of HBM traffic + 800 matmuls per
    core) disappears entirely.
  - Self-loops are synthetic identity-matmul tiles (lhsT=own rows,
    rhs=identity) seeding each chunk's PSUM accumulator - no gather
    descriptors, no separate self-term passes.
  - dis_src is folded into the gather-table rows (xs host-side; h2s rows
    scaled during the table build), dis_dst is applied per-chunk after the
    W matmul: the one-hot is a bare is_eq for BOTH layers (v3 spent
    ~290us/layer on the dissrc multiply).
  - Gather calls up to 2048 idxs (129 ring descs vs capacity 2048/queue);
    v3's 1024 split doubled the per-call overhead for nothing.
  - ONE collective: BN stats ride the AllGather of transposed agg1 (as v3).
    L2 self rows are rebuilt from the private ag_in copy (post-stats), so
    the h2s build needs no per-core control flow.
  - PSUM pools are bank-granular (8 banks): scoped per phase.

Sharding: nodes padded 100000 -> 102400 = 8*12800, core i owns rows
[i*12800,(i+1)*12800); edges partitioned by dst owner; weights replicated.
"""
import numpy as np
import ml_dtypes

import concourse.bass as bass
import concourse.mybir as mybir
import concourse.tile as tile
from concourse import bacc
from concourse.bass_utils import run_bass_kernel_spmd
from concourse.masks import make_identity

N = 100000
F = 128
NCORES = 8
NPAD = 102400
OWN = NPAD // NCORES          # 12800
CHUNKS = OWN // 128           # 100
GCHUNKS = NPAD // 128         # 800
NBLK = 4
BLK = NPAD // NBLK            # 25600 (< 32768, int16-addressable)
SEG = OWN + 128               # 12928 AllGather segment rows (stats header)
BN_EPS = 1e-5
SC = 2                        # dst chunks per super-chunk
QCAP = 1024                   # max idxs per gather call (65 ring descs)
BF16 = ml_dtypes.bfloat16

LAST_EXEC_NS = None
LAST_RESULT = None
_cache = {}


def _prep(x, edge_index):
    src = np.asarray(edge_index[0]).astype(np.int64)
    dst = np.asarray(edge_index[1]).astype(np.int64)

    deg = np.bincount(dst, minlength=N).astype(np.float32) + 1.0
    dis = np.zeros(NPAD, dtype=np.float32)
    dis[:N] = 1.0 / np.sqrt(deg)

    xs = np.zeros((NPAD, F), dtype=np.float32)
    xs[:N] = np.asarray(x, dtype=np.float32) * dis[:N, None]
    xs_tab = np.ascontiguousarray(xs.astype(BF16))         # [NPAD, F] bf16

    owner = dst // OWN
    chunk = (dst % OWN) // 128
    blk = src // BLK
    cell = ((owner * CHUNKS + chunk) * NBLK + blk).astype(np.int64)
    order = np.lexsort((src, cell))      # ascending src within each cell
    src_s = src[order]
    dst_s = dst[order]

    counts = np.zeros((NCORES, CHUNKS, NBLK), np.int64)
    np.add.at(counts, (owner, chunk, blk), 1)
    C = counts.max(axis=0)
    C = ((C + 127) // 128) * 128         # zero cells stay zero

    starts = np.zeros(NCORES * CHUNKS * NBLK + 1, dtype=np.int64)
    starts[1:] = np.cumsum(counts.reshape(-1))

    # super-chunk slot layout: for each super s: for each block b: the SC
    # cells (c, b) back to back.  Gather call = one (s, b) segment, split
    # to <=QCAP idxs.
    nsup = CHUNKS // SC
    slot_pos = {}
    sup_meta = []
    off = 0
    for s in range(nsup):
        chs = list(range(s * SC, (s + 1) * SC))
        sup_off = off
        seg_calls = []
        for b in range(NBLK):
            call_off = off
            for c in chs:
                slot_pos[(c, b)] = off
                off += int(C[c, b])
            seg_n = off - call_off
            sub = 0
            while sub < seg_n:
                n = min(QCAP, seg_n - sub)
                seg_calls.append((b, call_off + sub, n))
                sub += n
        chunk_of = []
        for b in range(NBLK):
            for ci, c in enumerate(chs):
                chunk_of.extend([ci] * (int(C[c, b]) // 128))
        last = {}
        for t, ci in enumerate(chunk_of):
            last[ci] = t
        sup_meta.append({"off": sup_off, "ntiles": len(chunk_of),
                         "chunk_of": chunk_of, "last": last,
                         "calls": seg_calls, "chunks": chs})
    tot = off
    ntiles = tot // 128

    per_core = []
    for i in range(NCORES):
        srcidx = np.zeros(tot, dtype=np.int16)                # pads hit row 0
        dstloc = np.full(tot, -1.0, dtype=np.float32)         # pads no column
        for c in range(CHUNKS):
            for b in range(NBLK):
                k = (i * CHUNKS + c) * NBLK + b
                m = int(counts[i, c, b])
                if m:
                    o = slot_pos[(c, b)]
                    sl = slice(starts[k], starts[k] + m)
                    srcidx[o:o + m] = (src_s[sl] - b * BLK).astype(np.int16)
                    dstloc[o:o + m] = (dst_s[sl] % 128).astype(np.float32)
        iw = srcidx.reshape(tot // 16, 16).T                  # [16, tot/16]
        srcidx_w = np.ascontiguousarray(np.tile(iw, (8, 1)))  # [128, tot/16]
        dstloc_t = np.ascontiguousarray(
            dstloc.reshape(ntiles, 128).T.astype(BF16))
        disT = np.ascontiguousarray(
            dis[i * OWN:(i + 1) * OWN].reshape(CHUNKS, 128).T)
        xs_own = np.ascontiguousarray(xs_tab[i * OWN:(i + 1) * OWN])
        per_core.append({"srcidx": srcidx_w, "dstloc": dstloc_t,
                         "disT": disT, "xs_own": xs_own})

    disG = np.ascontiguousarray(dis.reshape(GCHUNKS, 128).T)  # [128, 800]

    consts = {"tot": tot, "ntiles": ntiles, "sup_meta": sup_meta}
    return consts, xs_tab, disG, per_core


def _build(consts):
    tot = consts["tot"]
    ntiles = consts["ntiles"]
    sup_meta = consts["sup_meta"]

    f32 = mybir.dt.float32
    bf16 = mybir.dt.bfloat16
    AF = mybir.ActivationFunctionType
    OP = mybir.AluOpType
    nc = bacc.Bacc("TRN2", target_bir_lowering=False, debug=False,
                   num_devices=NCORES, num_swdge_queues=4,
                   dynamic_dma_scratch_size=32768)

    xstab_d = nc.dram_tensor("xs_tab", [NPAD, F], bf16, kind="ExternalInput").ap()
    xsown_d = nc.dram_tensor("xs_own", [OWN, F], bf16, kind="ExternalInput").ap()
    W1_d = nc.dram_tensor("W1b", [F, F], bf16, kind="ExternalInput").ap()
    W2_d = nc.dram_tensor("W2b", [F, F], bf16, kind="ExternalInput").ap()
    gamma_d = nc.dram_tensor("gamma_c", [F, 1], f32, kind="ExternalInput").ap()
    beta_d = nc.dram_tensor("beta_c", [F, 1], f32, kind="ExternalInput").ap()
    b2m_d = nc.dram_tensor("b2_mat", [128, F], f32, kind="ExternalInput").ap()
    disT_d = nc.dram_tensor("disT", [128, CHUNKS], f32, kind="ExternalInput").ap()
    disG_d = nc.dram_tensor("disG", [128, GCHUNKS], f32, kind="ExternalInput").ap()
    srcidx_d = nc.dram_tensor("srcidx", [128, tot // 16], mybir.dt.int16,
                              kind="ExternalInput").ap()
    dstloc_d = nc.dram_tensor("dstloc", [128, ntiles], bf16,
                              kind="ExternalInput").ap()
    out_d = nc.dram_tensor("out", [OWN, F], f32, kind="ExternalOutput").ap()

    h2s = nc.dram_tensor("h2s_tab", [NPAD, F], bf16)
    ag_in = nc.dram_tensor("ag_in", [SEG, F], bf16)
    ag_out = nc.dram_tensor("ag_out", [NCORES * SEG, F], bf16,
                            addr_space="Shared")

    with tile.TileContext(nc) as tc:
        with tc.tile_pool(name="const", bufs=1) as constp, \
             tc.tile_pool(name="big", bufs=1) as bigp, \
             tc.tile_pool(name="h", bufs=3) as hp, \
             tc.tile_pool(name="gbuf", bufs=3) as gbufp, \
             tc.tile_pool(name="oh", bufs=6) as ohp, \
             tc.tile_pool(name="wk", bufs=4) as wp, \
             tc.tile_pool(name="st", bufs=1) as stp:

            # ---- constants ----
            W1_t = constp.tile([F, F], bf16)
            W2_t = constp.tile([F, F], bf16)
            ident_f = constp.tile([128, 128], f32)
            ident_b = constp.tile([128, 128], bf16)
            iota4 = constp.tile([128, 4, 128], bf16)
            ones_f = constp.tile([128, 1], f32)
            ones_b = constp.tile([128, 1], bf16)
            gamma_t = constp.tile([F, 1], f32)
            beta_t = constp.tile([F, 1], f32)
            b2m_t = constp.tile([128, F], f32)
            disT_t = constp.tile([128, CHUNKS], f32)
            disG_t = constp.tile([128, GCHUNKS], f32)
            nc.sync.dma_start(out=W1_t[:], in_=W1_d[:])
            nc.sync.dma_start(out=W2_t[:], in_=W2_d[:])
            nc.sync.dma_start(out=gamma_t[:], in_=gamma_d[:])
            nc.sync.dma_start(out=beta_t[:], in_=beta_d[:])
            nc.sync.dma_start(out=b2m_t[:], in_=b2m_d[:])
            nc.sync.dma_start(out=disT_t[:], in_=disT_d[:])
            nc.sync.dma_start(out=disG_t[:], in_=disG_d[:])
            make_identity(nc, ident_f[:])
            make_identity(nc, ident_b[:])
            iota_i = constp.tile([128, 128], mybir.dt.int32)
            nc.gpsimd.iota(iota_i[:], pattern=[[1, 128]], base=0,
                           channel_multiplier=0)
            for k in range(4):
                nc.vector.tensor_copy(out=iota4[:, k, :], in_=iota_i[:])
            nc.vector.memset(ones_f[:], 1.0)
            nc.vector.memset(ones_b[:], 1.0)

            srcidx_sb = bigp.tile([128, tot // 16], mybir.dt.int16)
            dstloc_sb = bigp.tile([128, ntiles], bf16)
            xsown_sb = bigp.tile([128, CHUNKS, 128], bf16)
            aown_sb = bigp.tile([128, CHUNKS, 128], bf16)
            nc.sync.dma_start(out=srcidx_sb[:], in_=srcidx_d[:])
            nc.sync.dma_start(out=dstloc_sb[:], in_=dstloc_d[:])
            nc.sync.dma_start(
                out=xsown_sb[:],
                in_=xsown_d.rearrange("(k p) f -> p k f", p=128))

            # ---- shared gather/scatter pass (super-chunk granularity) ----
            # acc_c[f, d] = sum_e table[src_e, f] * onehot[e, d] + own[d, f]
            def layer_pass(table_ap, own_sb, psS, super_stage):
                qn = 0
                for sm in sup_meta:
                    TS = sm["ntiles"]
                    gb = gbufp.tile([128, max(TS, 1), 128], bf16, tag="gb")
                    base_t = sm["off"] // 128
                    for (b, coff, n) in sm["calls"]:
                        ol = coff - sm["off"]
                        nc.gpsimd.dma_gather(
                            gb[:, ol // 128:(ol + n) // 128, :],
                            table_ap[b * BLK:(b + 1) * BLK, :],
                            srcidx_sb[:, coff // 16:(coff + n) // 16],
                            n, n, F, queue_num=qn)
                        qn = (qn + 1) % 4
                    accs = [psS.tile([128, F], f32, tag="acc",
                                     name=f"acc{k}")
                            for k in range(SC)]
                    # self-loop seed: acc_c = own_rows_c^T (identity one-hot)
                    for ci, c in enumerate(sm["chunks"]):
                        nc.tensor.matmul(out=accs[ci][:],
                                         lhsT=own_sb[:, c, :],
                                         rhs=ident_b[:],
                                         start=True,
                                         stop=(ci not in sm["last"]))
                    t = 0
                    while t < TS:
                        w = min(4, TS - t)
                        oh = ohp.tile([128, 4, 128], bf16, tag="oh")
                        nc.vector.tensor_tensor(
                            out=oh[:, :w, :],
                            in0=dstloc_sb[:, base_t + t:base_t + t + w]
                                .to_broadcast([128, w, 128]),
                            in1=iota4[:, :w, :], op=OP.is_equal)
                        for j in range(w):
                            ci = sm["chunk_of"][t + j]
                            nc.tensor.matmul(out=accs[ci][:],
                                             lhsT=gb[:, t + j, :],
                                             rhs=oh[:, j, :],
                                             start=False,
                                             stop=(sm["last"][ci] == t + j))
                        t += w
                    super_stage(sm, accs)

            # ================= L1 pass (scoped PSUM pools) =================
            with tc.tile_pool(name="psS1", bufs=SC, space="PSUM") as psS1, \
                 tc.tile_pool(name="psW1", bufs=2, space="PSUM") as psW1, \
                 tc.tile_pool(name="psT1", bufs=2, space="PSUM") as psT1, \
                 tc.tile_pool(name="pss", bufs=1, space="PSUM") as pss:

                # BN stat accumulators (separate banks)
                sum_ps = pss.tile([128, 1], f32, name="sum_ps")
                sq_ps = pss.tile([128, 1], f32, name="sq_ps")

                # ---- L1: acc -> @W1 -> *dis_dst -> stats + transp. ship ----
                def l1_stage(sm, accs):
                    trp = psT1.tile([128, SC, 128], f32, tag="a")
                    for ci, c in enumerate(sm["chunks"]):
                        tc_sb = wp.tile([128, 128], bf16, tag="tc")
                        nc.scalar.activation(tc_sb[:], accs[ci][:], AF.Copy)
                        ps2 = psW1.tile([128, 128], f32, tag="w")
                        nc.tensor.matmul(out=ps2[:], lhsT=tc_sb[:],
                                         rhs=W1_t[:], start=True, stop=True)
                        asb = wp.tile([128, 128], f32, tag="asb")
                        nc.vector.tensor_scalar_mul(out=asb[:], in0=ps2[:],
                                                    scalar1=disT_t[:, c:c + 1])
                        nc.tensor.matmul(out=sum_ps[:], lhsT=asb[:],
                                         rhs=ones_f[:],
                                         start=(c == 0), stop=(c == CHUNKS - 1))
                        sq = wp.tile([128, 128], bf16, tag="sq")
                        nc.scalar.square(sq[:], asb[:])
                        nc.tensor.matmul(out=sq_ps[:], lhsT=sq[:],
                                         rhs=ones_b[:],
                                         start=(c == 0), stop=(c == CHUNKS - 1))
                        nc.tensor.transpose(out=trp[:, ci, :], in_=asb[:],
                                            identity=ident_f[:])
                    tst = wp.tile([128, SC, 128], bf16, tag="tT")
                    nc.scalar.activation(tst[:], trp[:], AF.Copy)
                    r0 = 128 + sm["chunks"][0] * 128
                    nc.sync.dma_start(
                        out=ag_in[r0:r0 + SC * 128, :]
                            .rearrange("(k p) f -> p k f", p=128),
                        in_=tst[:])

                layer_pass(xstab_d, xsown_sb, psS1, l1_stage)

                # ---- stats header -> ag_in rows [0,128) (rows 0,1 used) ----
                stats2 = stp.tile([128, 2], f32)
                nc.vector.tensor_copy(out=stats2[:, 0:1], in_=sum_ps[:])
                nc.vector.tensor_copy(out=stats2[:, 1:2], in_=sq_ps[:])
                stpad = stp.tile([128, 128], f32)
                nc.vector.memset(stpad[:], 0.0)
                nc.vector.tensor_copy(out=stpad[:, 0:2], in_=stats2[:])
                trs = psT1.tile([128, SC, 128], f32, tag="a")
                nc.tensor.transpose(out=trs[:, 0, :], in_=stpad[:],
                                    identity=ident_f[:])
                stag = stp.tile([128, 128], bf16)
                nc.scalar.activation(stag[:], trs[:, 0, :], AF.Copy)
                nc.sync.dma_start(out=ag_in[0:128, :], in_=stag[:])

            nc.gpsimd.collective_compute(
                "AllGather", OP.bypass, ins=[ag_in.ap()], outs=[ag_out.ap()],
                replica_groups=[list(range(NCORES))])

            # ================= mid phase (scoped PSUM pools) ===============
            with tc.tile_pool(name="psG", bufs=1, space="PSUM") as psG, \
                 tc.tile_pool(name="psAB", bufs=3, space="PSUM") as psAB:

                # ---- global BN stats from the 8 headers ----
                gst = stp.tile([16, 128], bf16)
                agv = ag_out.ap().rearrange("(i s) f -> i s f", i=NCORES)
                nc.sync.dma_start(out=gst[0:8, :], in_=agv[:, 0, :])
                nc.sync.dma_start(out=gst[8:16, :], in_=agv[:, 1, :])
                gpad = stp.tile([128, 128], f32)
                nc.vector.memset(gpad[:], 0.0)
                nc.vector.tensor_copy(out=gpad[0:16, :], in_=gst[:])
                gtr = psG.tile([128, 128], f32)
                nc.tensor.transpose(out=gtr[:], in_=gpad[:],
                                    identity=ident_f[:])
                # cols 0..7 = per-core sums, 8..15 = per-core sumsqs
                gred = stp.tile([128, 16], f32)
                nc.vector.tensor_copy(out=gred[:], in_=gtr[:, 0:16])
                nc.vector.tensor_tensor(out=gred[:, 0:4], in0=gred[:, 0:4],
                                        in1=gred[:, 4:8], op=OP.add)
                nc.vector.tensor_tensor(out=gred[:, 8:12], in0=gred[:, 8:12],
                                        in1=gred[:, 12:16], op=OP.add)
                nc.vector.tensor_tensor(out=gred[:, 0:2], in0=gred[:, 0:2],
                                        in1=gred[:, 2:4], op=OP.add)
                nc.vector.tensor_tensor(out=gred[:, 8:10], in0=gred[:, 8:10],
                                        in1=gred[:, 10:12], op=OP.add)
                nc.vector.tensor_tensor(out=gred[:, 0:1], in0=gred[:, 0:1],
                                        in1=gred[:, 1:2], op=OP.add)
                nc.vector.tensor_tensor(out=gred[:, 8:9], in0=gred[:, 8:9],
                                        in1=gred[:, 9:10], op=OP.add)

                mean_t = stp.tile([128, 1], f32)
                ex2_t = stp.tile([128, 1], f32)
                var_t = stp.tile([128, 1], f32)
                sd_t = stp.tile([128, 1], f32)
                rstd_t = stp.tile([128, 1], f32)
                scale_c = stp.tile([128, 1], f32)
                shift_c = stp.tile([128, 1], f32)
                eps_t = stp.tile([128, 1], f32)
                nc.vector.tensor_scalar_mul(out=mean_t[:], in0=gred[:, 0:1],
                                            scalar1=1.0 / N)
                nc.vector.tensor_scalar_mul(out=ex2_t[:], in0=gred[:, 8:9],
                                            scalar1=1.0 / N)
                nc.vector.tensor_tensor(out=var_t[:], in0=mean_t[:],
                                        in1=mean_t[:], op=OP.mult)
                nc.vector.tensor_tensor(out=var_t[:], in0=ex2_t[:],
                                        in1=var_t[:], op=OP.subtract)
                nc.vector.tensor_scalar_max(out=var_t[:], in0=var_t[:],
                                            scalar1=0.0)
                nc.vector.memset(eps_t[:], BN_EPS)
                nc.scalar.activation(sd_t[:], var_t[:], AF.Sqrt, bias=eps_t[:])
                nc.vector.reciprocal(out=rstd_t[:], in_=sd_t[:])
                nc.vector.tensor_tensor(out=scale_c[:], in0=rstd_t[:],
                                        in1=gamma_t[:], op=OP.mult)
                nc.vector.tensor_tensor(out=shift_c[:], in0=mean_t[:],
                                        in1=scale_c[:], op=OP.mult)
                nc.vector.tensor_tensor(out=shift_c[:], in0=beta_t[:],
                                        in1=shift_c[:], op=OP.subtract)

                # ---- L2 self rows: a_own*dis from the private ag_in copy ----
                for q in range(CHUNKS // 4):
                    stb = hp.tile([128, 4, 128], bf16, tag="cb")
                    nc.sync.dma_start(
                        out=stb[:],
                        in_=ag_in.ap()[128 + q * 512:128 + (q + 1) * 512, :]
                            .rearrange("(k p) f -> p k f", p=128))
                    h2a = wp.tile([128, 4, 128], f32, tag="h2")
                    nc.scalar.activation(h2a[:], stb[:], AF.Relu,
                                         bias=shift_c[:], scale=scale_c[:])
                    trp = psAB.tile([128, 4, 128], f32, tag="ab")
                    for j in range(4):
                        nc.tensor.transpose(out=trp[:, j, :], in_=h2a[:, j, :],
                                            identity=ident_f[:])
                    for j in range(4):
                        c = q * 4 + j
                        if j % 2 == 0:
                            nc.scalar.activation(aown_sb[:, c, :],
                                                 trp[:, j, :], AF.Copy,
                                                 scale=disT_t[:, c:c + 1])
                        else:
                            nc.vector.tensor_scalar_mul(
                                out=aown_sb[:, c, :], in0=trp[:, j, :],
                                scalar1=disT_t[:, c:c + 1])

                # ---- h2s table: relu(bn(agg1))*dis, node-major, all nodes ----
                for i in range(NCORES):
                    for g0 in range(0, CHUNKS, 4):
                        ctb = hp.tile([128, 4, 128], bf16, tag="cb")
                        rows0 = i * SEG + 128 + g0 * 128
                        nc.sync.dma_start(
                            out=ctb[:],
                            in_=ag_out.ap()[rows0:rows0 + 512, :]
                                .rearrange("(k p) f -> p k f", p=128))
                        h2a = wp.tile([128, 4, 128], f32, tag="h2")
                        nc.scalar.activation(h2a[:], ctb[:], AF.Relu,
                                             bias=shift_c[:], scale=scale_c[:])
                        trp = psAB.tile([128, 4, 128], f32, tag="ab")
                        for j in range(4):
                            nc.tensor.transpose(out=trp[:, j, :],
                                                in_=h2a[:, j, :],
                                                identity=ident_f[:])
                        hb2 = hp.tile([128, 4, F], bf16, tag="h")
                        for j in range(4):
                            g = i * CHUNKS + g0 + j
                            if j % 2 == 0:
                                nc.scalar.activation(hb2[:, j, :],
                                                     trp[:, j, :], AF.Copy,
                                                     scale=disG_t[:, g:g + 1])
                            else:
                                nc.vector.tensor_scalar_mul(
                                    out=hb2[:, j, :], in0=trp[:, j, :],
                                    scalar1=disG_t[:, g:g + 1])
                        orow = i * OWN + g0 * 128
                        nc.sync.dma_start(
                            out=h2s[orow:orow + 512, :]
                                .rearrange("(k p) f -> p k f", p=128),
                            in_=hb2[:])

            # ================= L2 pass (scoped PSUM pools) =================
            with tc.tile_pool(name="psS2", bufs=SC, space="PSUM") as psS2, \
                 tc.tile_pool(name="psW2", bufs=2, space="PSUM") as psW2:

                # ---- L2: acc -> @W2 -> *dis_dst -> +b2 -> relu -> out ----
                def l2_stage(sm, accs):
                    o1 = wp.tile([128, SC, 128], f32, tag="o1")
                    for ci, c in enumerate(sm["chunks"]):
                        tc_sb = wp.tile([128, 128], bf16, tag="tc")
                        nc.scalar.activation(tc_sb[:], accs[ci][:], AF.Copy)
                        ps2 = psW2.tile([128, 128], f32, tag="w")
                        nc.tensor.matmul(out=ps2[:], lhsT=tc_sb[:],
                                         rhs=W2_t[:], start=True, stop=True)
                        asb = wp.tile([128, 128], f32, tag="asb")
                        nc.vector.tensor_scalar_mul(out=asb[:], in0=ps2[:],
                                                    scalar1=disT_t[:, c:c + 1])
                        nc.vector.tensor_tensor(out=o1[:, ci, :], in0=asb[:],
                                                in1=b2m_t[:], op=OP.add)
                    ot = wp.tile([128, SC, 128], f32, tag="ot")
                    nc.scalar.activation(ot[:], o1[:], AF.Relu)
                    r0 = sm["chunks"][0] * 128
                    nc.sync.dma_start(
                        out=out_d[r0:r0 + SC * 128, :]
                            .rearrange("(k p) f -> p k f", p=128),
                        in_=ot[:])

                layer_pass(h2s.ap(), aown_sb, psS2, l2_stage)

    nc.compile()
    return nc


def kernel(**inputs):
    global LAST_EXEC_NS, LAST_RESULT
    import os
    x = inputs["x"]
    W1 = np.asarray(inputs["W1"], dtype=np.float32)
    W2 = np.asarray(inputs["W2"], dtype=np.float32)
    gamma = np.asarray(inputs["gamma"], dtype=np.float32)
    beta = np.asarray(inputs["beta"], dtype=np.float32)
    b2 = np.asarray(inputs["b2"], dtype=np.float32)
    edge_index = inputs["edge_index"]

    key = (hash(np.asarray(edge_index)[:, ::997].tobytes()),)
    if key not in _cache:
        consts, xs_tab, disG, per_core = _prep(x, edge_index)
        nc = _build(consts)
        _cache[key] = (consts, nc)
    else:
        consts, nc = _cache[key]
        _, xs_tab, disG, per_core = _prep(x, edge_index)

    shared = {
        "xs_tab": xs_tab,
        "disG": disG,
        "W1b": W1.astype(BF16), "W2b": W2.astype(BF16),
        "gamma_c": gamma.reshape(F, 1).copy(),
        "beta_c": beta.reshape(F, 1).copy(),
        "b2_mat": np.ascontiguousarray(np.broadcast_to(b2.reshape(1, F),
                                                       (128, F))),
    }
    in_maps = []
    for i in range(NCORES):
        m = dict(shared)
        m.update(per_core[i])
        in_maps.append(m)

    trace = bool(os.environ.get("BASS_GCN_TRACE"))
    res = run_bass_kernel_spmd(nc, in_maps, list(range(NCORES)), trace=trace)
    LAST_EXEC_NS = res.exec_time_ns
    LAST_RESULT = res

    out = np.concatenate([res.results[i]["out"] for i in range(NCORES)], axis=0)
    return np.ascontiguousarray(out[:N]).astype(np.float32)
